# revision 66
# baseline (speedup 1.0000x reference)
"""Causal self-attention with relative position encoding on 8 Trainium2 NeuronCores.

Problem: B=4, T=1024, C=256, H=8, E=32.
  q,k,v = x@W{q,k,v}.T ; att = q.k + einsum('qjhe,bhqe->bhqj', rel, k) ; scaled,
  causal-masked softmax ; y = att@v ; out = y@Wo.T

Sharding: query-row interleave across 8 cores (core m owns q = m+8t, t in [0,128)).

v4 design (vs v2 baseline, 148.6us):
 - prologue compression: consolidated one-shot input DMAs, critical loads
   (xq, Wk8, x8, rel0) on the sync ring, rest on scalar; rel0 split into 4
   per-g chunk tiles; head 0 runs groups small-first; PE warmup dummies;
   khat memsets split across gpsimd/vector.
 - rel prefetch 2 heads ahead (removes per-head-boundary DMA stalls).
 - K projection via 4-way col-tiled matmuls (tile_position=(0,32b), fp8
   non-DR, one 32-col Wk slice per head) so K psum lands directly in the
   KT4a (4b x 32e) partition layout — eliminates the KTst staging and the
   32 scatter dma_starts whose issue cost (~22us) poisoned a ring.
 - Q projection in bf16 (was fp8): halves the score quantization error.
 - V stays bf16 (fp8 V measured 3.3e-2 rel err — over the 2e-2 gate).
 - output in bf16, chunked across sync+scalar rings (was 512KB f32 on one
   queue at 22GB/s = 11us tail); host upcasts.
"""
import os
import numpy as np

import concourse.bass as bass
import concourse.mybir as mybir
import concourse.tile as tile

F32 = mybir.dt.float32
BF16 = mybir.dt.bfloat16
FP8 = mybir.dt.float8e4

B, T, C, H, E = 4, 1024, 256, 8, 32
NC = 8           # cores
TQ = T // NC     # 128 q rows per core
NG = 4           # row groups of 32 q rows
SCALE = 1.0 / np.sqrt(E)
NEG = -1.0e30
RSCL = 64.0      # rel & Q pre-scale
KSCL = 32.0      # K pre-scale (fp8 weight range); exp applies SCALE/(RSCL*KSCL)
VSCL = 32.0      # Wv fp8 pre-scale; folded out via WoT = Wo.T/VSCL


def EXT(g, kp2):
    """causal width of rel pass (g, kp2): q-rows tl' in [8*kp2, 8*kp2+8)."""
    return 256 * g + 64 * (kp2 + 1)


PER_G = [2 * (1024 * g + 640) for g in range(NG)]    # fp8 cols per (h,g)
PER_H = sum(PER_G)                                   # 17408
GBASE = [sum(PER_G[:g]) for g in range(NG)]


def _bloff(g, kp2):
    """col offset of pass (g,kp2) inside a per-h rel block."""
    return GBASE[g] + sum(2 * EXT(g, k) for k in range(kp2))


def rel_chunks(ext):
    """split [0,ext) at 512 boundaries (psum bank / moving-dim limits)."""
    if ext <= 512:
        return [(0, ext)]
    return [(0, 512), (512, ext)]


def sanitize_waits(nc):
    """This container's walrus accepts at most ONE sync wait per instruction.
    Hoist extra waits onto same-engine NOPs placed immediately before."""
    n = 0
    for f in nc.m.functions:
        for bb in f.blocks:
            new = []
            for inst in bb.instructions:
                si = inst.sync_info
                if si is not None and si.on_wait and len(si.on_wait) > 1:
                    waits = list(si.on_wait)
                    for w in waits[:-1]:
                        n += 1
                        nop = mybir.InstNoOp(
                            name=f"{inst.name}-sw{n}",
                            engine=inst.engine,
                            sync_info=mybir.SyncInfo(on_wait=[w], on_update=[]),
                            bass_nofuse=True,
                        )
                        new.append(nop)
                    si.on_wait = waits[-1:]
                new.append(inst)
            bb.instructions[:] = new
    return n


def build_program(sanitize=True):
    nc = bass.Bass("TRN2")
    relp_d = nc.dram_tensor("relp", [128, H * PER_H], FP8, kind="ExternalInput")
    # pass1 inputs, one dma: [0:1024) xq (col=256b+128half+t),
    # [1024:1536) WqT (col=256half+co), [1536:2048) WkT (col=256half+co)
    p1p_d = nc.dram_tensor("p1p", [128, 2048], BF16, kind="ExternalInput")
    # host-prepacked: col = 4096*nk + 1024*b + 512*ci_half + (t % 512)
    xTp_d = nc.dram_tensor("xTp", [128, 8192], BF16, kind="ExternalInput")
    WvT_d = nc.dram_tensor("WvT", [C, C], BF16, kind="ExternalInput")
    WoT_d = nc.dram_tensor("WoT", [C, C], BF16, kind="ExternalInput")
    msk_d = nc.dram_tensor("msk", [128, 256], F32, kind="ExternalInput")
    # out col = 256b + c, row = local q-row t
    out_d = nc.dram_tensor("out", [128, B * C], BF16, kind="ExternalOutput")

    with tile.TileContext(nc) as tc:
        with (
            tc.tile_pool(name="persist", bufs=1) as pp,
        ):
            # ---- persistent sbuf tensors ----
            # warmup stationary: memset (fast) so dummy matmuls start ASAP
            wmm = pp.tile([128, 128], BF16, tag="wmm", name="wmm")
            nc.gpsimd.memset(wmm[:], 0.0)
            ident = pp.tile([128, 128], BF16, tag="ident", name="ident")
            from concourse.masks import make_identity
            make_identity(nc, ident[:])

            # --- input loads (host-prepacked layouts -> 2-16KB dma lines) ---
            # sync ring: pass1 inputs (one dma), then xT half1, then rel
            w_sb = {}
            p1p = pp.tile([128, 2048], BF16, tag="p1p", name="p1p")
            nc.sync.dma_start(p1p[:], p1p_d[:, :])
            xqTall = p1p  # cols [0:1024)

            def wk_sl(p, c0, c1):
                return p1p[:, 1536 + 256 * p + c0:1536 + 256 * p + c1]

            xTall = pp.tile([128, 8192], BF16, tag="xTall", name="xTall")
            nc.sync.dma_start(xTall[:, 0:4096], xTp_d[:, 0:4096])

            def x_sl(b, ch, j0, j1):
                base = 4096 * (j0 // 512) + 1024 * b + 512 * ch + (j0 % 512)
                return xTall[:, base:base + (j1 - j0)]

            # scalar ring: WvT, xT j-half2, WoT, mask
            for half in range(2):
                t_ = pp.tile([128, 256], BF16, tag=f"WvT{half}", name=f"WvT{half}")
                nc.scalar.dma_start(t_[:], WvT_d[128 * half:128 * half + 128, :])
                w_sb[("WvT", half)] = t_
            nc.scalar.dma_start(xTall[:, 4096:8192], xTp_d[:, 4096:8192])
            for half in range(2):
                t_ = pp.tile([128, 256], BF16, tag=f"WoT{half}", name=f"WoT{half}")
                nc.scalar.dma_start(t_[:], WoT_d[128 * half:128 * half + 128, :])
                w_sb[("WoT", half)] = t_
            msk = pp.tile([128, 256], F32, tag="msk", name="msk")
            nc.scalar.dma_start(msk[:], msk_d[:])

            # rel head 0: four per-g chunk tiles (content g can start on chunk g)
            rel0g = []
            for g in range(NG):
                t_ = pp.tile([128, PER_G[g]], FP8, tag=f"rel0g{g}", name=f"rel0g{g}")
                nc.sync.dma_start(t_[:], relp_d[:, GBASE[g]:GBASE[g] + PER_G[g]])
                rel0g.append(t_)

            # K^T at the core's own q columns (khat source, fp8): [c-half, 128b + t]
            kqT = [pp.tile([128, 512], FP8, tag=f"kqT{i}", name=f"kqT{i}") for i in range(2)]
            # Q^T (x64) at core's q columns: [c-half, 128b + t]
            QT = [pp.tile([128, 512], BF16, tag=f"QT{i}", name=f"QT{i}") for i in range(2)]
            # batch-stacked K^T all heads: KT4a[32b+e, 1024h + j] (x32 scale)
            KT4a = pp.tile([128, H * T], BF16, tag="KT4a", name="KT4a")
            # batch-stacked V per 128-j block: V4[blk][j, 128h + 32b + e]
            V4 = [pp.tile([128, 1024], BF16, tag=f"V4_{b}", name=f"V4_{b}") for b in range(8)]
            # y^T accumulator: yT[hi][32hh + e, 128b + t]
            yT = [pp.tile([128, 512], BF16, tag=f"yT{i}", name=f"yT{i}") for i in range(2)]
            # block-diagonal stationaries (pre-zeroed once; nonzero slots
            # rewritten per head): khat fp8 [128, 4224], Q4 bf16 [128, 512]
            khat = [pp.tile([128, 4224], FP8, tag=f"khat{i}", name=f"khat{i}") for i in range(3)]
            Q4 = [pp.tile([128, 512], BF16, tag=f"Q4_{i}", name=f"Q4_{i}") for i in range(3)]
            # khat memsets are ~4us on one engine; split each across 3 engines
            # gpsimd zeroes its khat halves up front; vector only does khat0's
            # half + the Q4s now — khat1/2 vector-halves are deferred past the
            # pass-1 psum casts so those aren't stuck behind 3.6us of memsets
            for i in range(3):
                nc.gpsimd.memset(khat[i][:, 0:2112], 0.0)
                nc.vector.memset(Q4[i][:], 0.0)
            nc.vector.memset(khat[0][:, 2112:4224], 0.0)

            # scratch for warmup + act-table preload
            wtmp = pp.tile([128, 1], F32, tag="wtmp", name="wtmp")
            nc.vector.memset(wtmp[:], 0.0)
            nc.scalar.activation(wtmp[:], wtmp[:],
                                 mybir.ActivationFunctionType.Exp,
                                 bias=0.0, scale=0.0)

            # ================= prologue: projections =================
            with tc.tile_pool(name="prjps", bufs=4, space="PSUM") as prjps:
                # --- PE warmup: dummy matmuls so HAM un-throttles during DMA wait
                wps = prjps.tile([128, 512], F32, tag="prj", name="warm")
                for wi in range(20):
                    nc.tensor.matmul(wps[:, 0:128], wmm[:], wmm[:],
                                     start=True, stop=True, skip_group_check=True)

                # --- pass 1: kq^T (x32 via WkT host scale, cast fp8 on copy)
                # and Q^T (x64), both bf16 from xqTall ---
                for wof, dstT in [(1536, kqT), (1024, QT)]:
                    for m in range(2):
                        ps = prjps.tile([128, 512], F32, tag="prj", name="prj")
                        for b in range(B):
                            for p in range(2):
                                nc.tensor.matmul(
                                    ps[:, 128 * b:128 * b + 128],
                                    p1p[:, wof + 256 * p + 128 * m:
                                        wof + 256 * p + 128 * m + 128],
                                    xqTall[:, 256 * b + 128 * p:256 * b + 128 * p + 128],
                                    start=(p == 0), stop=(p == 1))
                        if wof == 1536:
                            nc.vector.tensor_copy(dstT[m][:], ps[:])  # f32->fp8
                        else:
                            nc.scalar.copy(dstT[m][:], ps[:])
                # deferred khat1/2 vector-half zeroing (needed by builds(1/2),
                # which run ~5us later on gpsimd)
                nc.vector.memset(khat[1][:, 2112:4224], 0.0)
                nc.vector.memset(khat[2][:, 2112:4224], 0.0)

                # --- pass 2: K directly in KT4a layout via 4-way col tiling.
                # Per (h, nk): 8 matmuls — for each ci half, the 4 batches run
                # concurrently in distinct 32-col groups (tile_position), each
                # streaming its own xT slice; psum rows land at 32b+e.
                for nk in range(2):
                    for h in range(H):
                        ps = prjps.tile([128, 512], F32, tag="prj", name="prj")
                        for ch in range(2):
                            for b in range(B):
                                rhs = x_sl(b, ch, 512 * nk, 512 * nk + 512)
                                nc.tensor.matmul(ps[32 * b:32 * b + 32, :],
                                                 wk_sl(ch, 32 * h, 32 * h + 32),
                                                 rhs,
                                                 start=(ch == 0), stop=(ch == 1),
                                                 tile_position=(0, 32 * b),
                                                 skip_group_check=True)
                        dst = KT4a[:, 1024 * h + 512 * nk:1024 * h + 512 * nk + 512]
                        if (h + nk) % 2 == 0:
                            nc.vector.tensor_copy(dst, ps[:])
                        else:
                            nc.scalar.copy(dst, ps[:])

                # --- pass 3: V = x @ Wv^T -> V4 (b,e)-stacked per j-block ---
                for blk in range(8):
                    for b in range(B):
                        ps = prjps.tile([128, 512], F32, tag="prj", name="prj")
                        for kp in range(2):
                            nc.tensor.matmul(
                                ps[:, 0:256],
                                x_sl(b, kp, 128 * blk, 128 * blk + 128),
                                w_sb[("WvT", kp)][:], start=(kp == 0), stop=(kp == 1))
                        dst = bass.AP(V4[blk][:].tensor, 32 * b, [[1024, 128], [128, 8], [1, 32]])
                        src = bass.AP(ps[:].tensor, 0, [[512, 128], [32, 8], [1, 32]])
                        if (blk + b) % 2 == 0:
                            nc.vector.tensor_copy(dst, src)
                        else:
                            nc.scalar.copy(dst, src)

            # ================= main loop =================
            # software-pipelined over it = (h, g), PV/transpose stage lags
            # scores by SHIFT iterations so PE never waits on the softmax chain
            SHIFT = 4
            with (
                tc.tile_pool(name="rels", bufs=3) as relsp,
                tc.tile_pool(name="pp2", bufs=2 + SHIFT) as pp2,
                tc.tile_pool(name="pts", bufs=4) as ptsp,
                tc.tile_pool(name="stats", bufs=6) as stats,
                tc.tile_pool(name="scps", bufs=2, space="PSUM") as scps,
                tc.tile_pool(name="ptps", bufs=2, space="PSUM") as ptps,
                tc.tile_pool(name="ctxps", bufs=2, space="PSUM") as ctxps,
            ):
                state = {}

                def fetch_rel(h):
                    rels = relsp.tile([128, PER_H], FP8, tag="rels", name="rels")
                    nc.sync.dma_start(rels[:], relp_d[:, PER_H * h:PER_H * (h + 1)])
                    state[(h, 'rels')] = rels

                def rel_src(h, g, kp2):
                    """(tile, base col offset) for rel pass (h, g, kp2)."""
                    if h == 0:
                        return rel0g[g], _bloff(g, kp2) - GBASE[g], PER_G[g]
                    return state[(h, 'rels')], _bloff(g, kp2), PER_H

                fetch_rel(1)

                def build_stationaries(h):
                    """khat/Q4 block-diag builds for head h (emitted one head
                    ahead so the Pool engine has a full head of lead time)."""
                    hh, hi, hp = h % 4, h // 4, h % 3
                    # khat: k at core's q rows, block-diag fp8 [128, 4224];
                    # nonzero at (32jq+e, 1056g + 256kp2 + 128i + 32b + tl'),
                    # tl' = 8kp2+4i+jq ; src kqT col = 128b + 32g + tl'
                    for jq in range(4):
                        dst = bass.AP(khat[hp][:].tensor, 32 * jq * 4224 + jq,
                                      [[4224, 32], [264, 16], [132, 2], [32, 4]])
                        src = bass.AP(kqT[hi][:].tensor, (32 * hh) * 512 + jq,
                                      [[512, 32], [8, 16], [4, 2], [128, 4]])
                        nc.gpsimd.tensor_copy(dst, src)
                    # Q4: x64 q, block-diag bf16 [128, 512]; nonzero at
                    # (32b+e, 128g + 32b + tl); src QT col = 128b + 32g + tl
                    for b in range(B):
                        dst = bass.AP(Q4[hp][:].tensor, (32 * b) * 512 + 32 * b,
                                      [[512, 32], [128, 4], [1, 32]])
                        src = bass.AP(QT[hi][:].tensor, (32 * hh) * 512 + 128 * b,
                                      [[512, 32], [32, 4], [1, 32]])
                        nc.gpsimd.tensor_copy(dst, src)

                def GORD(h):
                    # head 0 ascends so (h0, g) only needs rel0 chunk g;
                    # later heads descend (big first, small tail drains)
                    return [0, 1, 2, 3] if h == 0 else [3, 2, 1, 0]

                def stage_scores(it):
                    h, gp = it // NG, it % NG
                    g = GORD(h)[gp]
                    hh, hi, hp = h % 4, h // 4, h % 3
                    if gp == 0 and h + 2 < H:
                        # prefetch rel two heads ahead (sync ring, ~2.2MB)
                        fetch_rel(h + 2)
                    if gp == 1 and 0 <= h < H - 2:
                        build_stationaries(h + 2)
                    eg = 256 * (g + 1)
                    SC = scps.tile([128, 1024], F32, tag="SC", name="SC")
                    # content first (start=True covers [0, eg))
                    for (c0, c1) in rel_chunks(eg):
                        nc.tensor.matmul(SC[:, c0:c1],
                                         Q4[hp][:, 128 * g:128 * g + 128],
                                         KT4a[:, 1024 * h + c0:1024 * h + c1],
                                         start=True, stop=False,
                                         skip_group_check=True)
                    # rel passes: fp8 DoubleRow, contraction 256 = 8 q-rows x 32 e
                    for kp2 in range(4):
                        ext = EXT(g, kp2)
                        rtile, bo, rowlen = rel_src(h, g, kp2)
                        lhsT = bass.AP(khat[hp][:].tensor, 1056 * g + 256 * kp2,
                                       [[4224, 128], [128, 2], [1, 128]])
                        last = (kp2 == 3)
                        chs = rel_chunks(ext)
                        for ci, (c0, c1) in enumerate(chs):
                            rhs = bass.AP(rtile[:].tensor, bo + c0,
                                          [[rowlen, 128], [ext, 2], [1, c1 - c0]])
                            nc.tensor.matmul(SC[:, c0:c1], lhsT, rhs,
                                             start=False,
                                             stop=(last and ci == len(chs) - 1),
                                             perf_mode=mybir.MatmulPerfMode.DoubleRow,
                                             skip_group_check=True)
                    state[it] = SC
                    if gp == NG - 1 and h >= 1:
                        state.pop((h, 'rels'))

                def stage_softmax(it):
                    h, gp = it // NG, it % NG
                    g = GORD(h)[gp]
                    eg = 256 * (g + 1)
                    SC = state.pop(it)
                    nc.vector.tensor_add(SC[:, 256 * g:256 * g + 256],
                                         SC[:, 256 * g:256 * g + 256], msk[:])
                    P = pp2.tile([128, 1024], BF16, tag="P", name="P")
                    sums = stats.tile([128, 1], F32, tag="sums", name="sums")
                    nc.scalar.activation(P[:, 0:eg], SC[:, 0:eg],
                                         mybir.ActivationFunctionType.Exp,
                                         bias=0.0, scale=SCALE / (RSCL * KSCL),
                                         accum_out=sums[:])
                    rec = stats.tile([128, 1], F32, tag="rec", name="rec")
                    nc.vector.reciprocal(rec[:], sums[:])
                    nc.vector.tensor_scalar_mul(P[:, 0:eg], P[:, 0:eg], rec[:])
                    state[(it, 'P')] = P

                def stage_pv(it):
                    h, gp = it // NG, it % NG
                    g = GORD(h)[gp]
                    hh, hi = h % 4, h // 4
                    eg = 256 * (g + 1)
                    P = state.pop((it, 'P'))
                    if gp == 0:
                        # one [128, 512] psum tile accumulates ctx for all 4 g
                        state[(h, 'ctx')] = ctxps.tile([128, 512], F32, tag="ctx", name="ctx")
                    ctx = state[(h, 'ctx')]
                    njb = eg // 128
                    for jj in range(0, njb, 4):
                        nw = min(4, njb - jj)
                        ptp = ptps.tile([128, 512], BF16, tag="PTp", name="PTp")
                        for u in range(nw):
                            nc.tensor.transpose(ptp[:, 128 * u:128 * u + 128],
                                                P[:, 128 * (jj + u):128 * (jj + u) + 128],
                                                ident[:])
                        pts = ptsp.tile([128, 512], BF16, tag="PTs", name="PTs")
                        if (jj // 4) % 2 == 0:
                            nc.vector.tensor_copy(pts[:, 0:128 * nw], ptp[:, 0:128 * nw])
                        else:
                            nc.scalar.copy(pts[:, 0:128 * nw], ptp[:, 0:128 * nw])
                        for u in range(nw):
                            jb = jj + u
                            nc.tensor.matmul(ctx[:, 128 * g:128 * g + 128],
                                             V4[jb][:, 128 * h:128 * h + 128],
                                             pts[:, 128 * u:128 * u + 128],
                                             start=(jb == 0), stop=(jb == njb - 1),
                                             skip_group_check=True)
                    if gp == NG - 1:
                        # diag blocks -> y^T: one [32, 4g x 32tl] copy per batch
                        # src col = 128g' + 32b + tl ; dst col = 128b + 32g' + tl
                        state.pop((h, 'ctx'))
                        for b in range(B):
                            dst = bass.AP(yT[hi][:].tensor,
                                          (32 * hh) * 512 + 128 * b,
                                          [[512, 32], [32, 4], [1, 32]])
                            src = bass.AP(ctx[:].tensor,
                                          (32 * b) * 512 + 32 * b,
                                          [[512, 32], [128, 4], [1, 32]])
                            if b % 2 == 0:
                                nc.vector.tensor_copy(dst, src)
                            else:
                                nc.scalar.copy(dst, src)

                NIT = H * NG
                build_stationaries(0)
                build_stationaries(1)
                for it in range(NIT + SHIFT):
                    if SHIFT <= it:
                        stage_pv(it - SHIFT)
                    if it < NIT:
                        stage_scores(it)
                        stage_softmax(it)

                # ================= output projection =================
                otall = pp.tile([128, B * C], BF16, tag="otall", name="otall")
                for b in range(B):
                    ps = scps.tile([128, 256], F32, tag="SC", name="SC")
                    for half in range(2):
                        nc.tensor.matmul(ps[:], yT[half][:, 128 * b:128 * b + 128],
                                         w_sb[("WoT", half)][:],
                                         start=(half == 0), stop=(half == 1))
                    if b % 2 == 0:
                        nc.vector.tensor_copy(otall[:, 256 * b:256 * b + 256], ps[:])
                    else:
                        nc.scalar.copy(otall[:, 256 * b:256 * b + 256], ps[:])
                    if b == 1:
                        # flush the first b-pair while b=2,3 still compute
                        nc.sync.dma_start(out_d[:, 0:512], otall[:, 0:512])
                nc.scalar.dma_start(out_d[:, 512:1024], otall[:, 512:1024])
    if sanitize:
        sanitize_waits(nc)
    return nc


def make_mask(m):
    msk = np.zeros((128, 256), np.float32)
    jj = np.arange(256)[None, :]
    tl = (np.arange(128) % 32)[:, None]
    msk[jj > m + 8 * tl] = NEG
    return msk


def pack_rel_all(rel):
    """fp8 rel packs for all cores: relp_all[m] = [128, H*PER_H].

    Block (h, g, kp2): [128, 2*ext] (i-major), element
    [32jq+e, i*ext + j] = 64*rel[m + 8*(32g + 8kp2 + 4i + jq), j, h, e].
    """
    import ml_dtypes
    r8 = (rel * RSCL).astype(ml_dtypes.float8_e4m3)      # [T, T, H, E]
    # q = 8t + m -> [t, m, j, h, e]
    rr = r8.reshape(TQ, NC, T, H, E)
    relp = np.empty((NC, 128, H * PER_H), ml_dtypes.float8_e4m3)
    for g in range(NG):
        for kp2 in range(4):
            ext = EXT(g, kp2)
            t0 = 32 * g + 8 * kp2
            blk = rr[t0:t0 + 8, :, :ext, :, :]           # [k=4i+jq, m, j, h, e]
            blk = blk.reshape(2, 4, NC, ext, H, E)       # [i, jq, m, j, h, e]
            blk = blk.transpose(2, 4, 1, 5, 0, 3)        # [m, h, jq, e, i, j]
            blk = np.ascontiguousarray(blk).reshape(NC, H, 128, 2 * ext)
            for h in range(H):
                o = PER_H * h + _bloff(g, kp2)
                relp[:, :, o:o + 2 * ext] = blk[:, h]
    return relp


def host_common(x, Wq, Wk, Wv, Wo):
    import ml_dtypes
    xT = np.ascontiguousarray(x.transpose(0, 2, 1))      # [B, C, T] f32
    # xTp[p, 4096nk + 1024b + 512half + tj] = xT[b, 128half + p, 512nk + tj]
    xTp = np.ascontiguousarray(
        xT.reshape(B, 2, 128, 2, 512).transpose(2, 3, 0, 1, 4)).reshape(128, 8192)
    def wpack(W, s):
        return np.ascontiguousarray(
            (np.asarray(W, np.float32).T * s).reshape(2, 128, C).transpose(1, 0, 2)
        ).reshape(128, 2 * C)
    return {
        "xTp": xTp.astype(ml_dtypes.bfloat16),
        "WvT": np.ascontiguousarray(np.asarray(Wv, np.float32).T).astype(ml_dtypes.bfloat16),
        "WoT": np.ascontiguousarray(np.asarray(Wo, np.float32).T).astype(ml_dtypes.bfloat16),
    }, xT, np.concatenate([wpack(Wq, RSCL), wpack(Wk, KSCL)], axis=1)


_CACHE = {}


def kernel(x, rel_encoding, Wq, Wk, Wv, Wo, unused=None, **_):
    x = np.asarray(x, np.float32)
    rel = np.asarray(rel_encoding, np.float32)
    if "ncs" not in _CACHE:
        _CACHE["ncs"] = build_program()
    nc = _CACHE["ncs"]

    import ml_dtypes
    com, xT, wqk = host_common(x, Wq, Wk, Wv, Wo)
    relp_all = pack_rel_all(rel)
    in_maps = []
    for m in range(NC):
        im = dict(com)
        xq = xT[:, :, m::NC]                             # [B, C, TQ] f32
        # xqTp[p, 256b + 128half + t] = xq[b, 128half + p, t]
        xqTp = np.ascontiguousarray(
            xq.reshape(B, 2, 128, TQ).transpose(2, 0, 1, 3)).reshape(128, 1024)
        im.update({"relp": relp_all[m], "msk": make_mask(m),
                   "p1p": np.concatenate([xqTp, wqk], axis=1).astype(ml_dtypes.bfloat16)})
        in_maps.append(im)

    from concourse.bass_utils import run_bass_kernel_spmd
    res = run_bass_kernel_spmd(
        nc, in_maps, core_ids=list(range(NC)),
        trace=bool(int(os.environ.get("KERNEL_TRACE", "0"))),
    )
    _CACHE["last_results"] = res
    full = np.empty((B, T, C), np.float32)
    for m in range(NC):
        o = np.asarray(res.results[m]["out"]).astype(np.float32)   # [128, 4*256]
        full[:, m::NC, :] = o.reshape(TQ, B, C).transpose(1, 0, 2)
    return full
